# revision 59
# baseline (speedup 1.0000x reference)
"""Expert-parallel sparse MoE kernel for Trainium2 (8 NeuronCores).

Reference model: dense MoE (every expert on every token) followed by a
top-2-sparse combine, residual add, and LayerNorm.  Mathematically only the
top-2 experts per token contribute to the output, so the kernel routes each
token to its top-2 experts and only computes those expert FFNs.

Sharding: expert-parallel.  Each of the 8 cores owns 8 of the 64 experts and
receives the tokens routed to them (all-to-all by routing, done host-side as
part of sharding).  The device streams the expert weights (the dominant HBM
traffic, ~16MB/core as fp8) and computes y_e = relu(x @ W1[e] + b1[e]) @
W2[e] for every routed token.  The host applies the gate weights + b2 during
the unshard/scatter, adds the residual, and normalizes.

Per-core experts are ordered by routed-token count (descending) and each
slot k gets a static capacity cap_k = max over cores of the k-th largest
count, rounded up to 16.  The Bass program is compiled per capacity profile
(cached), so the mm1/relu work tracks the actual routing load (~30% less
than a uniform worst-case capacity) while staying SPMD-uniform across
cores.  Tokens routed beyond a slot capacity (can only happen for counts
>128) fall back to exact host compute.
"""

import numpy as np
import ml_dtypes

B, S, D, H, E, TOPK = 2, 1024, 512, 2048, 64, 2
T = B * S
NCORES = 8
EPC = E // NCORES          # experts per core
DC = D // 128              # 4 contraction chunks for x @ W1
HC = H // 128              # 16 contraction chunks for h @ W2
EPS = 1e-5
BF16 = ml_dtypes.bfloat16

PROFILE = False            # set True (module-level) to capture an NTFF trace
LAST_RESULT = None         # BassKernelResults of the last run (for test.py)

# fp8 everywhere on-device: W1/W2 are scaled by WSCALE on the host and
# stored e4m3 (the descale folds into the host-side combine); x and the
# hidden activations ride fp8 too (mm2 runs in DoubleRow mode for 2x PE
# rate).  End-to-end absmax-rel error ~1.5e-3.
USE_FP8 = True
WSCALE = 16.0
FP8 = ml_dtypes.float8_e4m3fn

_NC_CACHE = {}


def _flush_y(nc, mybir, acts, p2p, y, off, cap, wdt, last=False):
    """PSUM -> SBUF (fp8, scaled down by WSCALE) -> HBM for one expert.

    Stores ride the SWDGE (gpsimd) queue: per-expert HWDGE stores stall
    the weight-issuing sync sequencer on data-ready waits.  The last store
    goes on the (by then idle) sync ring for its lower latency."""
    yt = acts.tile([cap, D], wdt, name="yt")
    nc.vector.tensor_scalar(yt[:], p2p[:], 1.0 / WSCALE, None,
                            mybir.AluOpType.mult)
    if last:
        nc.sync.dma_start(y[off:off + cap, :], yt[:])
    else:
        nc.gpsimd.dma_start(y[off:off + cap, :], yt[:])


def _build_bass(caps):
    """Build the per-core Bass/Tile program for a slot-capacity profile."""
    import concourse.bacc as bacc
    import concourse.mybir as mybir
    from concourse import tile

    assert USE_FP8, "mm2 uses fp8 DoubleRow; bf16 fallback not supported"
    TS = sum(caps)
    offs = np.cumsum([0] + list(caps))[:-1]

    nc = bacc.Bacc("TRN2", target_bir_lowering=False, debug=False,
                   num_devices=1)

    f32 = mybir.dt.float32
    wdt = mybir.dt.float8e4
    xt = nc.dram_tensor("xt", [128, DC, TS], wdt, kind="ExternalInput")
    # W1|W2 fused per expert-slot: [d-part, HC*DC*128 (w1, j-major) +
    # HC*D (w2)]
    w12 = nc.dram_tensor("w12", [EPC, 128, DC * H + HC * D], wdt,
                         kind="ExternalInput")
    b1 = nc.dram_tensor("b1", [128, EPC, HC], f32, kind="ExternalInput")
    y = nc.dram_tensor("y", [TS, D], wdt, kind="ExternalOutput")

    relu = mybir.ActivationFunctionType.Relu
    double_row = mybir.MatmulPerfMode.DoubleRow

    with tile.TileContext(nc) as tc:
        with (
            tc.tile_pool(name="wts", bufs=EPC) as wts,
            tc.tile_pool(name="acts", bufs=2) as acts,
            tc.tile_pool(name="cst", bufs=1) as cst,
            tc.tile_pool(name="ps1", bufs=6, space="PSUM") as ps1,
            tc.tile_pool(name="ps2", bufs=2, space="PSUM") as ps2,
        ):
            # Tokens + biases ride the ACT (scalar) HWDGE ring in parallel
            # with the weight stream on the sync ring (slot 0's tokens
            # first).  A dummy ReLU reading b1 advances the ACT engine past
            # the DMA sem and pays the activation-table load once, so
            # steady-state Activations carry only their PSUM wait.
            xtt = cst.tile([128, DC, TS], wdt, name="xtt")
            b1t = cst.tile([128, EPC, HC], f32, name="b1t")
            # ONE token transfer (2.2KB contiguous per-partition runs): a
            # sliced "slot-0 first" op has ~0.4KB runs, crawls for ~19us,
            # and its held HWDGE lane stalls the weight-issue sequencer
            nc.scalar.dma_start(xtt[:], xt[:])
            nc.scalar.dma_start(b1t[:], b1[:])
            scratch = cst.tile([128, 1], f32, name="scratch")
            nc.scalar.activation(scratch[:], b1t[:, 0, 0:1], relu,
                                 bias=b1t[:, 0, 0:1])

            # HAM warmup: ~3.6us of dummy matmuls on a zeroed tile while
            # the PE would otherwise idle-wait for the first weights (the
            # DMA ramp can't feed even cold-rate mm1).  One busy SHORT
            # window (4096 cyc @1.2GHz) un-throttles the PE clock gate
            # 4/8 -> 8/8, so ALL real matmuls run at 2.4GHz instead of the
            # first ~10us running at 1.2.
            dummy = cst.tile([128, 512], wdt, name="dummy")
            nc.gpsimd.memset(dummy[:], 0)
            pd = ps2.tile([128, 512], f32, name="p2")
            for _ in range(9):
                nc.tensor.matmul(pd[:], dummy[:, :128], dummy[:],
                                 start=True, stop=True,
                                 skip_group_check=True)

            alu_add = mybir.AluOpType.add
            alu_max = mybir.AluOpType.max
            prev = None
            for i in range(EPC):
                cap = int(caps[i])
                off = int(offs[i])
                # ALL weights stream on the SP (sync) HWDGE ring as full
                # 1MB pieces (8KB-per-partition contiguous runs stream at
                # the full ~425GB/s) -- single FIFO in PE-consumption
                # order; the sync engine does nothing else.  The last
                # expert's w2 ships in halves so its first DR pairs overlap
                # the second half's arrival.  bufs=EPC: no weight DMA ever
                # waits on a buffer release (full 16MB prefetch window).
                w1t = wts.tile([128, HC, DC, 128], wdt, name="w1t")
                w2t = wts.tile([128, HC, D], wdt, name="w2t")
                src1 = w12[i][:, :DC * H].rearrange(
                    "p (j c k) -> p j c k", j=HC, c=DC)
                src2 = w12[i][:, DC * H:].rearrange("p (c dd) -> p c dd", c=HC)
                if i == 0:
                    # small first piece -> mm1 starts ~9.5us (inside the DMA
                    # ramp) and the PE HAM warms early; the piece completes
                    # fast so its HWDGE lane frees quickly
                    q = HC // 4
                    nc.sync.dma_start(w1t[:, :q], src1[:, :q])
                    nc.sync.dma_start(w1t[:, q:], src1[:, q:])
                else:
                    nc.sync.dma_start(w1t[:], src1)
                if i == EPC - 1:
                    nc.sync.dma_start(w2t[:, :HC // 2, :], src2[:, :HC // 2, :])
                    nc.sync.dma_start(w2t[:, HC // 2:, :], src2[:, HC // 2:, :])
                else:
                    nc.sync.dma_start(w2t[:], src2)

                # h^T = relu(W1^T x^T + b1), produced [h, token] so the
                # second matmul can contract over h on the partition dim.
                # mm2 is a DoubleRow fp8 matmul over h-chunk PAIRS
                # (contraction 256, both operands fp8e4) -- ~2x ALU rate at
                # free-dim 512.  It is software-pipelined one expert behind
                # (expert i-1's eight DR matmuls spread between expert i's
                # mm1 groups, where the relu chain they depend on is long
                # finished), so the PE stream has no expert-boundary
                # bubble; the final expert self-interleaves its DR pairs
                # right after each odd relu to keep the kernel tail short.
                # The relu+bias alternates between ScalarE and VectorE to
                # halve the ACT serial cost; both write fp8e4 (values are
                # scaled by WSCALE, well under the TRN fp8 max of 240).
                ht = acts.tile([128, HC, cap], wdt, name="ht")
                p2 = ps2.tile([cap, D], f32, name="p2")
                last_expert = (i == EPC - 1)
                for j in range(HC):
                    p1 = ps1.tile([128, cap], f32, name="p1")
                    for c in range(DC):
                        nc.tensor.matmul(
                            p1[:],
                            w1t[:, j, c, :],
                            xtt[:, c, off:off + cap],
                            start=(c == 0),
                            stop=(c == DC - 1),
                        )
                    jj = j // 2
                    if j % 2 == 0:
                        nc.scalar.activation(ht[:, j, :], p1[:], relu,
                                             bias=b1t[:, i, j:j + 1])
                        if prev is not None:
                            hp, wp, p2p, opv, cpv = prev
                            nc.tensor.matmul(
                                p2p[:], hp[:, 2 * jj:2 * jj + 2, :],
                                wp[:, 2 * jj:2 * jj + 2, :],
                                start=(jj == 0), stop=(jj == HC // 2 - 1),
                                perf_mode=double_row,
                                skip_group_check=True)
                            if jj == HC // 2 - 1:
                                _flush_y(nc, mybir, acts, p2p, y, opv, cpv,
                                         wdt)
                    else:
                        nc.vector.tensor_scalar(
                            ht[:, j, :], p1[:], b1t[:, i, j:j + 1], 0.0,
                            alu_add, alu_max)
                        if last_expert:
                            nc.tensor.matmul(
                                p2[:], ht[:, j - 1:j + 1, :],
                                w2t[:, j - 1:j + 1, :],
                                start=(jj == 0), stop=(jj == HC // 2 - 1),
                                perf_mode=double_row,
                                skip_group_check=True)
                            if jj == HC // 2 - 1:
                                _flush_y(nc, mybir, acts, p2, y, off, cap,
                                         wdt, last=True)
                if not last_expert:
                    prev = (ht, w2t, p2, off, cap)

    # Bacc lowering: splits excess per-instruction sem waits onto
    # InstEventSemaphore, moves matmul waits onto ldweights, inserts
    # activation table loads -- required for walrus codegen (1 wait slot
    # per 64B ISA instruction).
    nc.compile()
    return nc


def _get_nc(caps):
    caps = tuple(int(c) for c in caps)
    if caps not in _NC_CACHE:
        _NC_CACHE[caps] = _build_bass(caps)
    return _NC_CACHE[caps]


def kernel(x, Wg, bg, W1, b1, W2, b2, gamma, beta):
    global LAST_RESULT
    x = np.asarray(x, np.float32)
    Wg = np.asarray(Wg, np.float32)
    bg = np.asarray(bg, np.float32)
    W1 = np.asarray(W1, np.float32)
    b1 = np.asarray(b1, np.float32)
    W2 = np.asarray(W2, np.float32)
    b2 = np.asarray(b2, np.float32)
    gamma = np.asarray(gamma, np.float32)
    beta = np.asarray(beta, np.float32)

    xf = x.reshape(T, D)

    # ---- gating: softmax over experts, top-2 (ties -> lower index, as top_k)
    logits = xf @ Wg + bg
    logits -= logits.max(-1, keepdims=True)
    probs = np.exp(logits)
    probs /= probs.sum(-1, keepdims=True)
    idx = np.argsort(-probs, axis=-1, kind="stable")[:, :TOPK]   # [T, K]
    vals = np.take_along_axis(probs, idx, axis=-1)               # [T, K]

    # ---- per-expert token lists (the all-to-all "sharding by routing")
    toks_per_e = []
    overflow = []  # (expert, token_ids) beyond slot cap -> host fallback
    for e in range(E):
        te = np.nonzero((idx == e).any(-1))[0]
        toks_per_e.append(te)

    # ---- slot assignment: per core, experts ordered by load (desc); slot
    # capacity = max count at that rank across cores, rounded up to 16
    # (DoubleRow needs the h-pair stride %16; also keeps DMA runs aligned)
    counts = np.array([len(t) for t in toks_per_e])
    order = np.zeros((NCORES, EPC), np.int64)
    for c in range(NCORES):
        eids = np.arange(c * EPC, (c + 1) * EPC)
        order[c] = eids[np.argsort(-counts[eids], kind="stable")]
    rank_max = counts[order].max(axis=0)                          # [EPC]
    caps = np.minimum(np.maximum((rank_max + 15) // 16 * 16, 16), 128)
    offs = np.cumsum(np.concatenate([[0], caps]))[:-1]
    TS = int(caps.sum())

    for c in range(NCORES):
        for k in range(EPC):
            e = order[c, k]
            if len(toks_per_e[e]) > caps[k]:
                overflow.append((e, toks_per_e[e][caps[k]:]))
                toks_per_e[e] = toks_per_e[e][:caps[k]]

    # ---- pack per-core device inputs (layouts match SBUF tiles exactly)
    # x is packed fp8e4 unscaled (|x| <~ 5.3 fits comfortably; OCP e4m3 and
    # TRN fp8e4 bit patterns agree below 240).
    wq = lambda a: (a * WSCALE).astype(FP8)
    # w1 stored j-major per partition: [p, j(HC), c(DC), k(128)] so h-range
    # splits of the DMA stay per-partition-contiguous
    w1h = wq(W1).reshape(E, DC, 128, HC, 128).transpose(0, 2, 3, 1, 4)
    w2h = wq(W2).reshape(E, HC, 128, D).transpose(0, 2, 1, 3)
    w12h = np.concatenate([w1h.reshape(E, 128, DC * H),
                           w2h.reshape(E, 128, HC * D)], axis=2)
    b1s = (b1 * WSCALE).reshape(E, HC, 128).transpose(0, 2, 1)    # [E,128,HC]

    in_maps = []
    for c in range(NCORES):
        xth = np.zeros((128, DC, TS), FP8)
        for k in range(EPC):
            te = toks_per_e[order[c, k]]
            if len(te):
                blk = xf[te].T.reshape(DC, 128, len(te)).transpose(1, 0, 2)
                xth[:, :, offs[k]:offs[k] + len(te)] = blk.astype(FP8)
        in_maps.append({
            "xt": xth,
            "w12": np.ascontiguousarray(w12h[order[c]]),
            "b1": np.ascontiguousarray(
                b1s[order[c]].transpose(1, 0, 2)),               # [128,EPC,HC]
        })

    # ---- run on the 8 cores
    from concourse.bass_utils import run_bass_kernel_spmd
    nc = _get_nc(caps)
    res = run_bass_kernel_spmd(nc, in_maps, list(range(NCORES)),
                               trace=PROFILE)
    LAST_RESULT = res

    # ---- unshard: scatter expert outputs back by routing, combine, LN
    out = xf.copy()
    for c in range(NCORES):
        yc = np.asarray(res.results[c]["y"], FP8).astype(np.float32)
        yc /= WSCALE                                              # [TS, D]
        for k in range(EPC):
            e = order[c, k]
            te = toks_per_e[e]
            if not len(te):
                continue
            k_of = (idx[te] == e).argmax(-1)
            w = vals[te, k_of]
            ye = yc[offs[k]:offs[k] + len(te)]
            out[te] += w[:, None] * (ye + b2[e])

    for e, te in overflow:  # only possible for counts > 128
        k_of = (idx[te] == e).argmax(-1)
        w = vals[te, k_of]
        h = np.maximum(xf[te] @ W1[e] + b1[e], 0.0)
        out[te] += w[:, None] * (h @ W2[e] + b2[e])

    mu = out.mean(-1, keepdims=True)
    var = ((out - mu) ** 2).mean(-1, keepdims=True)
    o = (out - mu) / np.sqrt(var + EPS) * gamma + beta
    return o.reshape(B, S, D).astype(np.float32)
